# revision 9
# baseline (speedup 1.0000x reference)
"""MoE FFN (8 experts, top-2) on 8 TRN2 NeuronCores, expert-parallel.

Strategy:
  - Host: router (fp64 logits -> softmax -> top-2 -> renormalized combine
    weights), gather each expert's assigned tokens, pad to a common
    capacity C (SPMD: one program, per-core inputs).
  - Core e: full SwiGLU FFN for expert e over its C tokens in bf16
    (full PE rate, FWL fast weight loads, half the DMA of fp32r),
    combine-weight scaling on device; outputs [C, 1024] bf16.
  - Host: scatter-add per-expert outputs back into [B, S, D].

Device kernel structure (single pass over all C tokens):
  Phase 1 (gate/up): x kept fully in SBUF; sweeps of <=3 token groups
    (512-col PSUM banks, 128-aligned widths), k-outer matmul ordering so
    one stationary (weight k-tile) feeds all groups of a sweep; SwiGLU
    into a resident bf16 hbuf [128,32,C].
  Phase 2 (down): token chunks of <=768 (6 PSUM banks), stream down_w
    512-wide; accumulate over 32 h-tiles; combine-weight scale on evac.

Layouts (host-prepared, DMA-friendly):
  xT   [8, 128, C]      x[idx].T split along d into 8 k-tiles (bf16)
  gw/uw[32, 128, 8, 128] gate/up ^T tiled: [h_tile][d_sub][k][h] (bf16)
  dw   [32, 128, 1024]  down^T tiled:     [h_tile][h_sub][dout] (bf16)
  cwT  [128, C/128]     combine weights, partition-major (f32)
"""
import sys, os
for p in ("/opt/trn_rl_repo", os.path.join(os.path.dirname(os.path.abspath(__file__)))):
    if p not in sys.path:
        sys.path.insert(0, p)
import numpy as np
import ml_dtypes

BF16 = ml_dtypes.bfloat16
D_MODEL = 1024
D_INNER = 4096
N_EXPERTS = 8
TOP_K = 2
H_TILES = D_INNER // 128  # 32
K_TILES = D_MODEL // 128  # 8


def _p1_groups(C: int):
    """Token groups for phase 1: <=512 wide (one PSUM bank), 128-aligned,
    no group thinner than 384 when C allows (keeps every matmul's stream
    time above the PE sequencer's per-pair decode floor)."""
    ng = -(-C // 512)
    nb = C // 128  # 128-blocks
    base = nb // ng
    rem = nb % ng
    return [128 * (base + (1 if i < rem else 0)) for i in range(ng)]


def _build_nc(C: int, reps: int = 1):
    import concourse.bass as bass
    import concourse.mybir as mybir
    import concourse.tile as tile
    from concourse import bacc
    from contextlib import nullcontext

    f32 = mybir.dt.float32
    bf16 = mybir.dt.bfloat16
    Silu = mybir.ActivationFunctionType.Silu

    assert C % 128 == 0
    groups = _p1_groups(C)
    assert sum(groups) == C and all(g <= 512 for g in groups)
    sweeps = [groups[i:i + 3] for i in range(0, len(groups), 3)]
    # phase-2 token chunks (<=768 = 6 sub-blocks of 128 = 6 PSUM banks)
    ch2 = [768] * (C // 768) + ([C % 768] if C % 768 else [])

    nc = bacc.Bacc(None, target_bir_lowering=False)
    xT_d = nc.dram_tensor("xT", [K_TILES, 128, C], bf16, kind="ExternalInput")
    gw_d = nc.dram_tensor("gw", [H_TILES, 128, K_TILES, 128], bf16, kind="ExternalInput")
    uw_d = nc.dram_tensor("uw", [H_TILES, 128, K_TILES, 128], bf16, kind="ExternalInput")
    dw_d = nc.dram_tensor("dw", [H_TILES, 128, D_MODEL], bf16, kind="ExternalInput")
    cw_d = nc.dram_tensor("cwT", [128, C // 128], f32, kind="ExternalInput")
    y_d = nc.dram_tensor("y", [C, D_MODEL], bf16, kind="ExternalOutput")

    with tile.TileContext(nc) as tc:
        with (
            tc.tile_pool(name="xt", bufs=1) as xt_pool,
            tc.tile_pool(name="wgt", bufs=4) as wgt_pool,
            tc.tile_pool(name="dwp", bufs=6) as dw_pool,
            tc.tile_pool(name="hb", bufs=1) as hb_pool,
            tc.tile_pool(name="sg", bufs=3) as sg_pool,
            tc.tile_pool(name="yo", bufs=4) as y_pool,
            tc.tile_pool(name="cw", bufs=1) as cw_pool,
            tc.tile_pool(name="ps", bufs=8, space="PSUM") as ps,
        ):
            cw_sb = cw_pool.tile([128, C // 128], f32)
            nc.sync.dma_start(cw_sb[:], cw_d[:])

            rep_ctx = (
                tc.For_i(0, reps, 1,
                         hint_engines=(mybir.EngineType.PE, mybir.EngineType.SP))
                if reps > 1 else nullcontext()
            )
            with rep_ctx:
                xt = xt_pool.tile([128, K_TILES, C], bf16, tag="xt")
                # per-k DMAs: first matmuls gate on fewer bytes
                for k in range(K_TILES):
                    nc.sync.dma_start(xt[:, k, :], xT_d[k])
                hbuf = hb_pool.tile([128, H_TILES, C], bf16, tag="hbuf")

                # ---- phase 1: gate/up + SwiGLU into hbuf
                t0 = 0
                for sw in sweeps:
                    sl = []
                    g0 = t0
                    for gsz in sw:
                        sl.append((g0, gsz))
                        g0 += gsz
                    for hi in range(H_TILES):
                        gw = wgt_pool.tile([128, K_TILES, 128], bf16, tag="gw")
                        nc.sync.dma_start(gw[:], gw_d[hi])
                        uw = wgt_pool.tile([128, K_TILES, 128], bf16, tag="uw")
                        nc.sync.dma_start(uw[:], uw_d[hi])
                        pg = [ps.tile([128, gsz], f32, tag="ps", name="pg",
                                      padded_shape=[128, 512]) for (_, gsz) in sl]
                        pu = [ps.tile([128, gsz], f32, tag="ps", name="pu",
                                      padded_shape=[128, 512]) for (_, gsz) in sl]
                        for k in range(K_TILES):
                            for j, (gs, gsz) in enumerate(sl):
                                nc.tensor.matmul(
                                    pg[j][:], gw[:, k, :], xt[:, k, gs:gs + gsz],
                                    start=(k == 0), stop=(k == K_TILES - 1))
                            for j, (gs, gsz) in enumerate(sl):
                                nc.tensor.matmul(
                                    pu[j][:], uw[:, k, :], xt[:, k, gs:gs + gsz],
                                    start=(k == 0), stop=(k == K_TILES - 1))
                        for j, (gs, gsz) in enumerate(sl):
                            sg = sg_pool.tile([128, gsz], f32, tag="sg", name="sg",
                                              padded_shape=[128, 512])
                            nc.scalar.activation(sg[:], pg[j][:], Silu)
                            nc.vector.tensor_mul(hbuf[:, hi, gs:gs + gsz], sg[:], pu[j][:])
                    t0 = g0

                # ---- phase 2: down-projection + combine-weight scale
                t0 = 0
                for TC in ch2:
                    n_sub = TC // 128
                    for half in range(2):
                        ds_ = slice(half * 512, (half + 1) * 512)
                        yp = [None] * n_sub
                        for hi in range(H_TILES):
                            dwt = dw_pool.tile([128, 512], bf16, tag="dw", name="dwt")
                            nc.sync.dma_start(dwt[:], dw_d[hi][:, ds_])
                            for s in range(n_sub):
                                ts_ = slice(t0 + s * 128, t0 + (s + 1) * 128)
                                if hi == 0:
                                    yp[s] = ps.tile([128, 512], f32, tag="ps", name="yp")
                                nc.tensor.matmul(
                                    yp[s][:], hbuf[:, hi, ts_], dwt[:],
                                    start=(hi == 0), stop=(hi == H_TILES - 1))
                        for s in range(n_sub):
                            gcol = t0 // 128 + s
                            yt = y_pool.tile([128, 512], bf16, tag="yt", name="yt")
                            nc.vector.tensor_scalar_mul(
                                yt[:], yp[s][:], cw_sb[:, gcol:gcol + 1])
                            nc.sync.dma_start(
                                y_d[t0 + s * 128: t0 + (s + 1) * 128, ds_], yt[:])
                    t0 += TC
    nc.finalize()
    return nc


_NC_CACHE: dict = {}


def _get_nc(C: int):
    if C not in _NC_CACHE:
        _NC_CACHE[C] = _build_nc(C)
    return _NC_CACHE[C]


def _route(x2d: np.ndarray, router_w: np.ndarray, router_b: np.ndarray):
    """fp64 router: returns (idx_per_expert, cw_per_expert) lists."""
    logits = x2d.astype(np.float64) @ router_w.astype(np.float64).T + router_b.astype(np.float64)
    m = logits.max(axis=-1, keepdims=True)
    p = np.exp(logits - m)
    p /= p.sum(axis=-1, keepdims=True)
    # top-2 (jax.lax.top_k picks largest; softmax is monotonic in logits)
    i1 = np.argmax(p, axis=-1)
    p_masked = p.copy()
    p_masked[np.arange(p.shape[0]), i1] = -1.0
    i2 = np.argmax(p_masked, axis=-1)
    p1 = p[np.arange(p.shape[0]), i1]
    p2 = p[np.arange(p.shape[0]), i2]
    denom = p1 + p2
    w1 = p1 / denom
    w2 = p2 / denom
    idxs, cws = [], []
    for e in range(N_EXPERTS):
        sel1 = np.nonzero(i1 == e)[0]
        sel2 = np.nonzero(i2 == e)[0]
        idx = np.concatenate([sel1, sel2])
        cw = np.concatenate([w1[sel1], w2[sel2]])
        idxs.append(idx)
        cws.append(cw.astype(np.float32))
    return idxs, cws


def _prep_core_inputs(x2d, idxs, cws, gate_w, up_w, down_w, C):
    in_maps = []
    for e in range(N_EXPERTS):
        idx = idxs[e]
        n = len(idx)
        xe = np.zeros((C, D_MODEL), np.float32)
        xe[:n] = x2d[idx]
        xT = np.ascontiguousarray(xe.T).astype(BF16).reshape(K_TILES, 128, C)
        gw = np.ascontiguousarray(
            gate_w[e].T.reshape(K_TILES, 128, H_TILES, 128).transpose(2, 1, 0, 3)).astype(BF16)
        uw = np.ascontiguousarray(
            up_w[e].T.reshape(K_TILES, 128, H_TILES, 128).transpose(2, 1, 0, 3)).astype(BF16)
        dw = np.ascontiguousarray(down_w[e].T).reshape(H_TILES, 128, D_MODEL).astype(BF16)
        cw = np.zeros((C,), np.float32)
        cw[:n] = cws[e]
        cwT = np.ascontiguousarray(cw.reshape(-1, 128).T)
        in_maps.append({"xT": xT, "gw": gw, "uw": uw, "dw": dw, "cwT": cwT})
    return in_maps


def kernel(x, router_w, router_b, gate_w, up_w, down_w):
    from concourse.bass_utils import run_bass_kernel_spmd

    x = np.asarray(x, dtype=np.float32)
    router_w = np.asarray(router_w, dtype=np.float32)
    router_b = np.asarray(router_b, dtype=np.float32)
    gate_w = np.asarray(gate_w, dtype=np.float32)
    up_w = np.asarray(up_w, dtype=np.float32)
    down_w = np.asarray(down_w, dtype=np.float32)

    B, S, D = x.shape
    x2d = x.reshape(B * S, D)
    idxs, cws = _route(x2d, router_w, router_b)
    max_n = max(len(i) for i in idxs)
    C = max(256, ((max_n + 127) // 128) * 128)

    nc = _get_nc(C)
    in_maps = _prep_core_inputs(x2d, idxs, cws, gate_w, up_w, down_w, C)
    res = run_bass_kernel_spmd(nc, in_maps, core_ids=list(range(N_EXPERTS)), trace=False)

    out = np.zeros((B * S, D_MODEL), np.float32)
    for e in range(N_EXPERTS):
        n = len(idxs[e])
        np.add.at(out, idxs[e], res.results[e]["y"][:n].astype(np.float32))
    return out.reshape(B, S, D_MODEL)


# revision 10
# speedup vs baseline: 1.0062x; 1.0062x over previous
"""MoE FFN (8 experts, top-2) on 8 TRN2 NeuronCores, expert-parallel.

Strategy:
  - Host: router (fp64 logits -> softmax -> top-2 -> renormalized combine
    weights), gather each expert's assigned tokens, pad to a common
    capacity C (SPMD: one program, per-core inputs).
  - Core e: full SwiGLU FFN for expert e over its C tokens in bf16
    (full PE rate, FWL fast weight loads, half the DMA of fp32r),
    combine-weight scaling on device; outputs [C, 1024] bf16.
  - Host: scatter-add per-expert outputs back into [B, S, D].

Device kernel structure (single pass over all C tokens):
  Phase 1 (gate/up): x kept fully in SBUF; sweeps of <=3 token groups
    (512-col PSUM banks, 128-aligned widths), k-outer matmul ordering so
    one stationary (weight k-tile) feeds all groups of a sweep; SwiGLU
    into a resident bf16 hbuf [128,32,C].
  Phase 2 (down): token chunks of <=768 (6 PSUM banks), stream down_w
    512-wide; accumulate over 32 h-tiles; combine-weight scale on evac.

Layouts (host-prepared, DMA-friendly):
  xT   [8, 128, C]      x[idx].T split along d into 8 k-tiles (bf16)
  gw/uw[32, 128, 8, 128] gate/up ^T tiled: [h_tile][d_sub][k][h] (bf16)
  dw   [32, 128, 1024]  down^T tiled:     [h_tile][h_sub][dout] (bf16)
  cwT  [128, C/128]     combine weights, partition-major (f32)
"""
import sys, os
for p in ("/opt/trn_rl_repo", os.path.join(os.path.dirname(os.path.abspath(__file__)))):
    if p not in sys.path:
        sys.path.insert(0, p)
import numpy as np
import ml_dtypes

BF16 = ml_dtypes.bfloat16
D_MODEL = 1024
D_INNER = 4096
N_EXPERTS = 8
TOP_K = 2
H_TILES = D_INNER // 128  # 32
K_TILES = D_MODEL // 128  # 8


def _p1_groups(C: int):
    """Token groups for phase 1: <=512 wide (one PSUM bank), 128-aligned.
    Full 512-wide groups plus one tail measured fastest on HW."""
    return [512] * (C // 512) + ([C % 512] if C % 512 else [])


def _build_nc(C: int, reps: int = 1):
    import concourse.bass as bass
    import concourse.mybir as mybir
    import concourse.tile as tile
    from concourse import bacc
    from contextlib import nullcontext

    f32 = mybir.dt.float32
    bf16 = mybir.dt.bfloat16
    Silu = mybir.ActivationFunctionType.Silu

    assert C % 128 == 0
    groups = _p1_groups(C)
    assert sum(groups) == C and all(g <= 512 for g in groups)
    sweeps = [groups[i:i + 3] for i in range(0, len(groups), 3)]
    # phase-2 token chunks (<=768 = 6 sub-blocks of 128 = 6 PSUM banks)
    ch2 = [768] * (C // 768) + ([C % 768] if C % 768 else [])

    nc = bacc.Bacc(None, target_bir_lowering=False)
    xT_d = nc.dram_tensor("xT", [K_TILES, 128, C], bf16, kind="ExternalInput")
    gw_d = nc.dram_tensor("gw", [H_TILES, 128, K_TILES, 128], bf16, kind="ExternalInput")
    uw_d = nc.dram_tensor("uw", [H_TILES, 128, K_TILES, 128], bf16, kind="ExternalInput")
    dw_d = nc.dram_tensor("dw", [H_TILES, 128, D_MODEL], bf16, kind="ExternalInput")
    cw_d = nc.dram_tensor("cwT", [128, C // 128], f32, kind="ExternalInput")
    y_d = nc.dram_tensor("y", [C, D_MODEL], bf16, kind="ExternalOutput")

    with tile.TileContext(nc) as tc:
        with (
            tc.tile_pool(name="xt", bufs=1) as xt_pool,
            tc.tile_pool(name="wgt", bufs=4) as wgt_pool,
            tc.tile_pool(name="dwp", bufs=6) as dw_pool,
            tc.tile_pool(name="hb", bufs=1) as hb_pool,
            tc.tile_pool(name="sg", bufs=3) as sg_pool,
            tc.tile_pool(name="yo", bufs=4) as y_pool,
            tc.tile_pool(name="cw", bufs=1) as cw_pool,
            tc.tile_pool(name="ps", bufs=8, space="PSUM") as ps,
        ):
            cw_sb = cw_pool.tile([128, C // 128], f32)
            nc.sync.dma_start(cw_sb[:], cw_d[:])

            rep_ctx = (
                tc.For_i(0, reps, 1,
                         hint_engines=(mybir.EngineType.PE, mybir.EngineType.SP))
                if reps > 1 else nullcontext()
            )
            with rep_ctx:
                xt = xt_pool.tile([128, K_TILES, C], bf16, tag="xt")
                # per-k DMAs: first matmuls gate on fewer bytes
                for k in range(K_TILES):
                    nc.sync.dma_start(xt[:, k, :], xT_d[k])
                hbuf = hb_pool.tile([128, H_TILES, C], bf16, tag="hbuf")

                # ---- phase 1: gate/up + SwiGLU into hbuf
                t0 = 0
                for sw in sweeps:
                    sl = []
                    g0 = t0
                    for gsz in sw:
                        sl.append((g0, gsz))
                        g0 += gsz
                    for hi in range(H_TILES):
                        gw = wgt_pool.tile([128, K_TILES, 128], bf16, tag="gw")
                        nc.sync.dma_start(gw[:], gw_d[hi])
                        uw = wgt_pool.tile([128, K_TILES, 128], bf16, tag="uw")
                        nc.sync.dma_start(uw[:], uw_d[hi])
                        pg = [ps.tile([128, gsz], f32, tag="ps", name="pg",
                                      padded_shape=[128, 512]) for (_, gsz) in sl]
                        pu = [ps.tile([128, gsz], f32, tag="ps", name="pu",
                                      padded_shape=[128, 512]) for (_, gsz) in sl]
                        for k in range(K_TILES):
                            for j, (gs, gsz) in enumerate(sl):
                                nc.tensor.matmul(
                                    pg[j][:], gw[:, k, :], xt[:, k, gs:gs + gsz],
                                    start=(k == 0), stop=(k == K_TILES - 1))
                            for j, (gs, gsz) in enumerate(sl):
                                nc.tensor.matmul(
                                    pu[j][:], uw[:, k, :], xt[:, k, gs:gs + gsz],
                                    start=(k == 0), stop=(k == K_TILES - 1))
                        for j, (gs, gsz) in enumerate(sl):
                            sg = sg_pool.tile([128, gsz], f32, tag="sg", name="sg",
                                              padded_shape=[128, 512])
                            nc.scalar.activation(sg[:], pg[j][:], Silu)
                            nc.vector.tensor_mul(hbuf[:, hi, gs:gs + gsz], sg[:], pu[j][:])
                    t0 = g0

                # ---- phase 2: down-projection + combine-weight scale
                t0 = 0
                for TC in ch2:
                    n_sub = TC // 128
                    for half in range(2):
                        ds_ = slice(half * 512, (half + 1) * 512)
                        yp = [None] * n_sub
                        for hi in range(H_TILES):
                            dwt = dw_pool.tile([128, 512], bf16, tag="dw", name="dwt")
                            nc.sync.dma_start(dwt[:], dw_d[hi][:, ds_])
                            for s in range(n_sub):
                                ts_ = slice(t0 + s * 128, t0 + (s + 1) * 128)
                                if hi == 0:
                                    yp[s] = ps.tile([128, 512], f32, tag="ps", name="yp")
                                nc.tensor.matmul(
                                    yp[s][:], hbuf[:, hi, ts_], dwt[:],
                                    start=(hi == 0), stop=(hi == H_TILES - 1))
                        for s in range(n_sub):
                            gcol = t0 // 128 + s
                            yt = y_pool.tile([128, 512], bf16, tag="yt", name="yt")
                            nc.vector.tensor_scalar_mul(
                                yt[:], yp[s][:], cw_sb[:, gcol:gcol + 1])
                            nc.sync.dma_start(
                                y_d[t0 + s * 128: t0 + (s + 1) * 128, ds_], yt[:])
                    t0 += TC
    nc.finalize()
    return nc


_NC_CACHE: dict = {}


def _get_nc(C: int):
    if C not in _NC_CACHE:
        _NC_CACHE[C] = _build_nc(C)
    return _NC_CACHE[C]


def _route(x2d: np.ndarray, router_w: np.ndarray, router_b: np.ndarray):
    """fp64 router: returns (idx_per_expert, cw_per_expert) lists."""
    logits = x2d.astype(np.float64) @ router_w.astype(np.float64).T + router_b.astype(np.float64)
    m = logits.max(axis=-1, keepdims=True)
    p = np.exp(logits - m)
    p /= p.sum(axis=-1, keepdims=True)
    # top-2 (jax.lax.top_k picks largest; softmax is monotonic in logits)
    i1 = np.argmax(p, axis=-1)
    p_masked = p.copy()
    p_masked[np.arange(p.shape[0]), i1] = -1.0
    i2 = np.argmax(p_masked, axis=-1)
    p1 = p[np.arange(p.shape[0]), i1]
    p2 = p[np.arange(p.shape[0]), i2]
    denom = p1 + p2
    w1 = p1 / denom
    w2 = p2 / denom
    idxs, cws = [], []
    for e in range(N_EXPERTS):
        sel1 = np.nonzero(i1 == e)[0]
        sel2 = np.nonzero(i2 == e)[0]
        idx = np.concatenate([sel1, sel2])
        cw = np.concatenate([w1[sel1], w2[sel2]])
        idxs.append(idx)
        cws.append(cw.astype(np.float32))
    return idxs, cws


def _prep_core_inputs(x2d, idxs, cws, gate_w, up_w, down_w, C):
    in_maps = []
    for e in range(N_EXPERTS):
        idx = idxs[e]
        n = len(idx)
        xe = np.zeros((C, D_MODEL), np.float32)
        xe[:n] = x2d[idx]
        xT = np.ascontiguousarray(xe.T).astype(BF16).reshape(K_TILES, 128, C)
        gw = np.ascontiguousarray(
            gate_w[e].T.reshape(K_TILES, 128, H_TILES, 128).transpose(2, 1, 0, 3)).astype(BF16)
        uw = np.ascontiguousarray(
            up_w[e].T.reshape(K_TILES, 128, H_TILES, 128).transpose(2, 1, 0, 3)).astype(BF16)
        dw = np.ascontiguousarray(down_w[e].T).reshape(H_TILES, 128, D_MODEL).astype(BF16)
        cw = np.zeros((C,), np.float32)
        cw[:n] = cws[e]
        cwT = np.ascontiguousarray(cw.reshape(-1, 128).T)
        in_maps.append({"xT": xT, "gw": gw, "uw": uw, "dw": dw, "cwT": cwT})
    return in_maps


def kernel(x, router_w, router_b, gate_w, up_w, down_w):
    from concourse.bass_utils import run_bass_kernel_spmd

    x = np.asarray(x, dtype=np.float32)
    router_w = np.asarray(router_w, dtype=np.float32)
    router_b = np.asarray(router_b, dtype=np.float32)
    gate_w = np.asarray(gate_w, dtype=np.float32)
    up_w = np.asarray(up_w, dtype=np.float32)
    down_w = np.asarray(down_w, dtype=np.float32)

    B, S, D = x.shape
    x2d = x.reshape(B * S, D)
    idxs, cws = _route(x2d, router_w, router_b)
    max_n = max(len(i) for i in idxs)
    C = max(256, ((max_n + 127) // 128) * 128)

    nc = _get_nc(C)
    in_maps = _prep_core_inputs(x2d, idxs, cws, gate_w, up_w, down_w, C)
    res = run_bass_kernel_spmd(nc, in_maps, core_ids=list(range(N_EXPERTS)), trace=False)

    out = np.zeros((B * S, D_MODEL), np.float32)
    for e in range(N_EXPERTS):
        n = len(idxs[e])
        np.add.at(out, idxs[e], res.results[e]["y"][:n].astype(np.float32))
    return out.reshape(B, S, D_MODEL)


# revision 12
# speedup vs baseline: 1.1325x; 1.1256x over previous
"""MoE FFN (8 experts, top-2) on 8 TRN2 NeuronCores, expert-parallel.

Strategy:
  - Host: router (fp64 logits -> softmax -> top-2 -> renormalized combine
    weights), gather each expert's assigned tokens, pad to a common
    capacity C (SPMD: one program, per-core inputs).
  - Core e: full SwiGLU FFN for expert e over its C tokens in bf16
    (full PE rate, FWL fast weight loads, half the DMA of fp32r),
    combine-weight scaling on device; outputs [C, 1024] bf16.
  - Host: scatter-add per-expert outputs back into [B, S, D].

Device kernel structure (single pass over all C tokens):
  Phase 1 (gate/up): x kept fully in SBUF; sweeps of <=3 token groups
    (512-col PSUM banks, 128-aligned widths), k-outer matmul ordering so
    one stationary (weight k-tile) feeds all groups of a sweep; SwiGLU
    into a resident bf16 hbuf [128,32,C].
  Phase 2 (down): token chunks of <=768 (6 PSUM banks), stream down_w
    512-wide; accumulate over 32 h-tiles; combine-weight scale on evac.

Layouts (host-prepared, DMA-friendly):
  xT   [8, 128, C]      x[idx].T split along d into 8 k-tiles (bf16)
  gw/uw[32, 128, 8, 128] gate/up ^T tiled: [h_tile][d_sub][k][h] (bf16)
  dw   [32, 128, 1024]  down^T tiled:     [h_tile][h_sub][dout] (bf16)
  cwT  [128, C/128]     combine weights, partition-major (f32)
"""
import sys, os
for p in ("/opt/trn_rl_repo", os.path.join(os.path.dirname(os.path.abspath(__file__)))):
    if p not in sys.path:
        sys.path.insert(0, p)
import numpy as np
import ml_dtypes

BF16 = ml_dtypes.bfloat16
D_MODEL = 1024
D_INNER = 4096
N_EXPERTS = 8
TOP_K = 2
H_TILES = D_INNER // 128  # 32
K_TILES = D_MODEL // 128  # 8


CAPACITY = 2048  # capacity-factor-1.0: perfect expert balance on device;
                 # the few overflow pairs are combined on the host in f32.


def _capacity(max_n: int) -> int:
    return max(256, min(CAPACITY, ((max_n + 127) // 128) * 128))


def _p1_groups(C: int):
    """Token groups for phase 1: <=512 wide (one PSUM bank), 128-aligned.
    Full 512-wide groups plus one tail measured fastest on HW."""
    return [512] * (C // 512) + ([C % 512] if C % 512 else [])


def _build_nc(C: int, reps: int = 1):
    import concourse.bass as bass
    import concourse.mybir as mybir
    import concourse.tile as tile
    from concourse import bacc
    from contextlib import nullcontext

    f32 = mybir.dt.float32
    bf16 = mybir.dt.bfloat16
    Silu = mybir.ActivationFunctionType.Silu

    assert C % 128 == 0
    groups = _p1_groups(C)
    assert sum(groups) == C and all(g <= 512 for g in groups)
    sweeps = [groups[i:i + 3] for i in range(0, len(groups), 3)]
    # phase-2 token chunks (<=768 = 6 sub-blocks of 128 = 6 PSUM banks)
    ch2 = [768] * (C // 768) + ([C % 768] if C % 768 else [])

    nc = bacc.Bacc(None, target_bir_lowering=False)
    xT_d = nc.dram_tensor("xT", [K_TILES, 128, C], bf16, kind="ExternalInput")
    gw_d = nc.dram_tensor("gw", [H_TILES, 128, K_TILES, 128], bf16, kind="ExternalInput")
    uw_d = nc.dram_tensor("uw", [H_TILES, 128, K_TILES, 128], bf16, kind="ExternalInput")
    dw_d = nc.dram_tensor("dw", [H_TILES, 128, D_MODEL], bf16, kind="ExternalInput")
    cw_d = nc.dram_tensor("cwT", [128, C // 128], f32, kind="ExternalInput")
    y_d = nc.dram_tensor("y", [C, D_MODEL], bf16, kind="ExternalOutput")

    with tile.TileContext(nc) as tc:
        with (
            tc.tile_pool(name="xt", bufs=1) as xt_pool,
            tc.tile_pool(name="wgt", bufs=4) as wgt_pool,
            tc.tile_pool(name="dwp", bufs=6) as dw_pool,
            tc.tile_pool(name="hb", bufs=1) as hb_pool,
            tc.tile_pool(name="sg", bufs=3) as sg_pool,
            tc.tile_pool(name="yo", bufs=4) as y_pool,
            tc.tile_pool(name="cw", bufs=1) as cw_pool,
            tc.tile_pool(name="ps", bufs=8, space="PSUM") as ps,
        ):
            cw_sb = cw_pool.tile([128, C // 128], f32)
            nc.sync.dma_start(cw_sb[:], cw_d[:])

            rep_ctx = (
                tc.For_i(0, reps, 1,
                         hint_engines=(mybir.EngineType.PE, mybir.EngineType.SP))
                if reps > 1 else nullcontext()
            )
            with rep_ctx:
                xt = xt_pool.tile([128, K_TILES, C], bf16, tag="xt")
                # per-k DMAs: first matmuls gate on fewer bytes
                for k in range(K_TILES):
                    nc.sync.dma_start(xt[:, k, :], xT_d[k])
                hbuf = hb_pool.tile([128, H_TILES, C], bf16, tag="hbuf")

                # ---- phase 1: gate/up + SwiGLU into hbuf
                t0 = 0
                for sw in sweeps:
                    sl = []
                    g0 = t0
                    for gsz in sw:
                        sl.append((g0, gsz))
                        g0 += gsz
                    for hi in range(H_TILES):
                        gw = wgt_pool.tile([128, K_TILES, 128], bf16, tag="gw")
                        nc.sync.dma_start(gw[:], gw_d[hi])
                        uw = wgt_pool.tile([128, K_TILES, 128], bf16, tag="uw")
                        nc.sync.dma_start(uw[:], uw_d[hi])
                        pg = [ps.tile([128, gsz], f32, tag="ps", name="pg",
                                      padded_shape=[128, 512]) for (_, gsz) in sl]
                        pu = [ps.tile([128, gsz], f32, tag="ps", name="pu",
                                      padded_shape=[128, 512]) for (_, gsz) in sl]
                        for k in range(K_TILES):
                            for j, (gs, gsz) in enumerate(sl):
                                nc.tensor.matmul(
                                    pg[j][:], gw[:, k, :], xt[:, k, gs:gs + gsz],
                                    start=(k == 0), stop=(k == K_TILES - 1))
                            for j, (gs, gsz) in enumerate(sl):
                                nc.tensor.matmul(
                                    pu[j][:], uw[:, k, :], xt[:, k, gs:gs + gsz],
                                    start=(k == 0), stop=(k == K_TILES - 1))
                        for j, (gs, gsz) in enumerate(sl):
                            sg = sg_pool.tile([128, gsz], f32, tag="sg", name="sg",
                                              padded_shape=[128, 512])
                            nc.scalar.activation(sg[:], pg[j][:], Silu)
                            nc.vector.tensor_mul(hbuf[:, hi, gs:gs + gsz], sg[:], pu[j][:])
                    t0 = g0

                # ---- phase 2: down-projection + combine-weight scale
                t0 = 0
                for TC in ch2:
                    n_sub = TC // 128
                    for half in range(2):
                        ds_ = slice(half * 512, (half + 1) * 512)
                        yp = [None] * n_sub
                        for hi in range(H_TILES):
                            dwt = dw_pool.tile([128, 512], bf16, tag="dw", name="dwt")
                            nc.sync.dma_start(dwt[:], dw_d[hi][:, ds_])
                            for s in range(n_sub):
                                ts_ = slice(t0 + s * 128, t0 + (s + 1) * 128)
                                if hi == 0:
                                    yp[s] = ps.tile([128, 512], f32, tag="ps", name="yp")
                                nc.tensor.matmul(
                                    yp[s][:], hbuf[:, hi, ts_], dwt[:],
                                    start=(hi == 0), stop=(hi == H_TILES - 1))
                        for s in range(n_sub):
                            gcol = t0 // 128 + s
                            yt = y_pool.tile([128, 512], bf16, tag="yt", name="yt")
                            nc.vector.tensor_scalar_mul(
                                yt[:], yp[s][:], cw_sb[:, gcol:gcol + 1])
                            nc.sync.dma_start(
                                y_d[t0 + s * 128: t0 + (s + 1) * 128, ds_], yt[:])
                    t0 += TC
    nc.finalize()
    return nc


_NC_CACHE: dict = {}


def _get_nc(C: int):
    if C not in _NC_CACHE:
        _NC_CACHE[C] = _build_nc(C)
    return _NC_CACHE[C]


def _route(x2d: np.ndarray, router_w: np.ndarray, router_b: np.ndarray):
    """fp64 router: returns (idx_per_expert, cw_per_expert) lists."""
    logits = x2d.astype(np.float64) @ router_w.astype(np.float64).T + router_b.astype(np.float64)
    m = logits.max(axis=-1, keepdims=True)
    p = np.exp(logits - m)
    p /= p.sum(axis=-1, keepdims=True)
    # top-2 (jax.lax.top_k picks largest; softmax is monotonic in logits)
    i1 = np.argmax(p, axis=-1)
    p_masked = p.copy()
    p_masked[np.arange(p.shape[0]), i1] = -1.0
    i2 = np.argmax(p_masked, axis=-1)
    p1 = p[np.arange(p.shape[0]), i1]
    p2 = p[np.arange(p.shape[0]), i2]
    denom = p1 + p2
    w1 = p1 / denom
    w2 = p2 / denom
    idxs, cws = [], []
    for e in range(N_EXPERTS):
        sel1 = np.nonzero(i1 == e)[0]
        sel2 = np.nonzero(i2 == e)[0]
        idx = np.concatenate([sel1, sel2])
        cw = np.concatenate([w1[sel1], w2[sel2]])
        idxs.append(idx)
        cws.append(cw.astype(np.float32))
    return idxs, cws


def _prep_core_inputs(x2d, idxs, cws, gate_w, up_w, down_w, C):
    in_maps = []
    for e in range(N_EXPERTS):
        idx = idxs[e]
        n = len(idx)
        xe = np.zeros((C, D_MODEL), np.float32)
        xe[:n] = x2d[idx]
        xT = np.ascontiguousarray(xe.T).astype(BF16).reshape(K_TILES, 128, C)
        gw = np.ascontiguousarray(
            gate_w[e].T.reshape(K_TILES, 128, H_TILES, 128).transpose(2, 1, 0, 3)).astype(BF16)
        uw = np.ascontiguousarray(
            up_w[e].T.reshape(K_TILES, 128, H_TILES, 128).transpose(2, 1, 0, 3)).astype(BF16)
        dw = np.ascontiguousarray(down_w[e].T).reshape(H_TILES, 128, D_MODEL).astype(BF16)
        cw = np.zeros((C,), np.float32)
        cw[:n] = cws[e]
        cwT = np.ascontiguousarray(cw.reshape(-1, 128).T)
        in_maps.append({"xT": xT, "gw": gw, "uw": uw, "dw": dw, "cwT": cwT})
    return in_maps


def _silu(v):
    return v / (1.0 + np.exp(-v))


def kernel(x, router_w, router_b, gate_w, up_w, down_w):
    from concourse.bass_utils import run_bass_kernel_spmd

    x = np.asarray(x, dtype=np.float32)
    router_w = np.asarray(router_w, dtype=np.float32)
    router_b = np.asarray(router_b, dtype=np.float32)
    gate_w = np.asarray(gate_w, dtype=np.float32)
    up_w = np.asarray(up_w, dtype=np.float32)
    down_w = np.asarray(down_w, dtype=np.float32)

    B, S, D = x.shape
    x2d = x.reshape(B * S, D)
    idxs, cws = _route(x2d, router_w, router_b)
    max_n = max(len(i) for i in idxs)
    C = _capacity(max_n)

    # device gets the first C pairs per expert; overflow handled on host
    dev_idxs = [i[:C] for i in idxs]
    dev_cws = [c[:C] for c in cws]

    nc = _get_nc(C)
    in_maps = _prep_core_inputs(x2d, dev_idxs, dev_cws, gate_w, up_w, down_w, C)
    res = run_bass_kernel_spmd(nc, in_maps, core_ids=list(range(N_EXPERTS)), trace=False)

    out = np.zeros((B * S, D_MODEL), np.float32)
    for e in range(N_EXPERTS):
        n = len(dev_idxs[e])
        np.add.at(out, dev_idxs[e], res.results[e]["y"][:n].astype(np.float32))
        if len(idxs[e]) > C:  # capacity overflow: combine on host in f32
            oi = idxs[e][C:]
            ocw = cws[e][C:]
            xs = x2d[oi]
            h = _silu(xs @ gate_w[e].T) * (xs @ up_w[e].T)
            np.add.at(out, oi, ocw[:, None] * (h @ down_w[e].T))
    return out.reshape(B, S, D_MODEL)
